# revision 55
# baseline (speedup 1.0000x reference)
"""Trainium2 Bass kernel for nn_BasisFunction2D (2-D basis-function embedding lookup).

Reformulation: data-dependent bilinear interpolation over a 16x16 grid of
per-(ix,iz) tables expressed as dense hat-function interpolation matrices

    V[(q,iz), b] = hat_q(z[iz,b])      (z-side weights, 2 nonzeros per column)
    U[(p,ix), b] = hat_p(x[ix,b])      (x-side weights)

with partition-of-unity folds on BOTH axes, which shrink the contraction to
K=512 (4 full PE chunks) and the free dim to M=512 (one PSUM bank per chain):

    out[o,b] = sum_m C_o[b,m] * U[m,b]  +  side[o,b]
    C_o[b,m] = sum_{k in 512} V[k,b] * Ghh_o[k,m]         (PE, bf16, N=512)
    side = exact fp32 rank-small correction evaluated on the host.

V and U are computed exactly on the host in fp32 and shipped as bf16 (no
on-device build).  The matmul schedule is o-pair-major:

    for pair (2 o's): for bc: for kc: for o' in pair: matmul

so consecutive matmuls alternate PSUM banks (216ns steady cadence) and each
chain drains straight out of PSUM on the DVE (fused multiply-reduce against
U, no ACT hop).  The start is DMA-latency-bound, so a small "boot" tensor
carries exactly what block 0 needs, G0..G3 ship as int8 (half the bytes
through the supply-critical first ~10us; upcast x gscale on the otherwise
idle DVE/ACT engines; the int8 rounding costs ~0.6% extra rel err on half
the outputs, total ~1.2e-2 vs the 2e-2 gate), and G4..G7 ship as bf16 with
relaxed deadlines.  Dummy warmup matmuls ramp the PE p-state (0.65 -> 2.4
GHz needs ~3us of continuous busy) while the first chunks land.
"""

import numpy as np

import concourse.bass as bass
import concourse.bacc as bacc_mod
import concourse.tile as tile
from concourse import mybir
from concourse.bass_utils import run_bass_kernel_spmd

F32 = mybir.dt.float32
BF16 = mybir.dt.bfloat16
ALU = mybir.AluOpType

NCORES = 8
NG = 16            # grid bins
NQ = 17            # grid corners per axis
IX = 32
IZ = 32
OUT = 64
B = 512
OSH = OUT // NCORES          # outputs per core = 8
KF = NG * IZ                 # 512 folded contraction rows (q<=15, iz)
M = NG * IX                  # 512 folded free cols (p<=15, ix)
BIG = 1e30
NBC = B // 128               # 4 batch chunks
NKC = 4                      # contraction chunks of 128
NWARM = 8            # PE warmup matmuls (p-state ramp + DMA cover)

_NC_CACHE = {}


def _build_nc(gscale):
    """Build the single-core Bass/Tile program (identical across cores).
    gscale is the int8 dequant scale for the g8 slabs (compile-time const)."""
    nc = bacc_mod.Bacc(None, target_bir_lowering=False)
    gmain_d = nc.dram_tensor("gmain", [OSH, 128, NKC * M], BF16, kind="ExternalInput")
    # vq: V chunks for bc1..3 only (bc0 lives in boot), kc-major:
    # col ((kc*3 + (bc-1)) * 128)
    vq_d = nc.dram_tensor("vq", [128, NKC * 3 * 128], BF16, kind="ExternalInput")
    uu_d = nc.dram_tensor("uu", [128, NBC * M], BF16, kind="ExternalInput")
    # boot: [vq bc0 kc0..3 (4*128) | G0 kc0 (512) | G1 kc0 (512)] — one small
    # leading DMA so block 0 can start before the big tiles land.
    boot_d = nc.dram_tensor("boot", [128, 4 * 128 + 2 * M], BF16,
                            kind="ExternalInput")
    # g8: int8 slabs for the DMA-supply-critical tiles G0..G3 (kc0 of G0/G1
    # lives in boot as bf16).  Slab order by consumption deadline:
    # [G0kc1 G1kc1 G0kc2 G1kc2 G0kc3 G1kc3 G2kc0 G3kc0 ... G2kc3 G3kc3]
    g8_d = nc.dram_tensor("g8", [128, 14 * M], mybir.dt.int8,
                          kind="ExternalInput")
    out_d = nc.dram_tensor("out", [B, OSH], F32, kind="ExternalOutput")

    with tile.TileContext(nc) as tc:
        with (
            tc.tile_pool(name="per", bufs=1) as per,       # persistent tiles
            tc.tile_pool(name="junk", bufs=2) as junk,     # stt mandatory outs
            tc.tile_pool(name="ps", bufs=8, space="PSUM") as ps,
        ):
            # ---------------- PE warmup ----------------
            # Dependency-free dummy matmuls ramp the PE p-state while the
            # first input DMAs land.  The memset rides the otherwise-idle
            # DVE queue.
            wt = per.tile([128, B], BF16, tag="warm", name="wt")
            nc.vector.memset(wt[:], 0.0)
            wps = ps.tile([128, B], F32, tag="ps", name="wps")
            for _ in range(NWARM):
                nc.tensor.matmul(wps[:], wt[:, 0:128], wt[:], start=True, stop=True)
            for _ in range(4):
                nc.tensor.matmul(wps[:, 0:128], wt[:, 0:128], wt[:, 0:128],
                                 start=True, stop=True)

            # ---------------- input loads ----------------
            # Start-critical chunks lead the sync/scalar queues (gpsimd's
            # queue is stalled by framework drains); first matmul needs only
            # vq[:, :128] + G0's first half.  U tiles ride gpsimd (needed
            # ~5us later than the G stream).
            vqt = per.tile([128, NKC * 3 * 128], BF16, tag="vq", name="vqt")
            U_sb = [per.tile([128, M], BF16, tag=f"U{bc}", name=f"U{bc}")
                    for bc in range(NBC)]
            G_sb = [per.tile([128, NKC * M], BF16, tag=f"G{o}", name=f"G{o}")
                    for o in range(OSH)]

            # ordered by consumption deadline; g8 (int8, half the bytes)
            # carries the supply-critical G0..G3 slabs and is upcast on the
            # DVE (early slabs, before drains start) and ACT (late slabs).
            boot = per.tile([128, 4 * 128 + 2 * M], BF16, tag="boot", name="boot")
            g8t = per.tile([128, 14 * M], mybir.dt.int8, tag="g8", name="g8t")
            VB = 3 * 128
            # late G tiles ride the gpsimd queue (its framework drains clear
            # by ~16us, well before the pair-2/3 deadlines), freeing sync/
            # scalar bandwidth for the supply-critical first 12us.
            nc.scalar.dma_start(g8t[:, 0:4 * M], g8_d[:, 0:4 * M])
            nc.sync.dma_start(boot[:], boot_d[:, :])
            nc.scalar.dma_start(g8t[:, 4 * M:6 * M], g8_d[:, 4 * M:6 * M])
            nc.sync.dma_start(vqt[:, 0:2 * VB], vq_d[:, 0:2 * VB])
            nc.sync.dma_start(vqt[:, 2 * VB:4 * VB], vq_d[:, 2 * VB:4 * VB])
            nc.sync.dma_start(U_sb[0][:], uu_d[:, 0:M])
            nc.sync.dma_start(g8t[:, 6 * M:10 * M], g8_d[:, 6 * M:10 * M])
            nc.sync.dma_start(g8t[:, 10 * M:14 * M], g8_d[:, 10 * M:14 * M])
            nc.scalar.dma_start(U_sb[1][:], uu_d[:, M:2 * M])
            nc.scalar.dma_start(U_sb[2][:], uu_d[:, 2 * M:3 * M])
            nc.scalar.dma_start(U_sb[3][:], uu_d[:, 3 * M:4 * M])
            nc.sync.dma_start(G_sb[4][:], gmain_d[4])
            nc.scalar.dma_start(G_sb[5][:], gmain_d[5])
            nc.sync.dma_start(G_sb[6][:], gmain_d[6])
            nc.scalar.dma_start(G_sb[7][:], gmain_d[7])

            # int8 -> bf16 upcasts (x gscale); slab s holds (o_loc, kc):
            SLABS = [(0, 1), (1, 1), (0, 2), (1, 2), (0, 3), (1, 3),
                     (2, 0), (3, 0), (2, 1), (3, 1), (2, 2), (3, 2),
                     (2, 3), (3, 3)]
            for s, (o, kc) in enumerate(SLABS):
                dst = G_sb[o][:, kc * M:(kc + 1) * M]
                src = g8t[:, s * M:(s + 1) * M]
                if s < 4:
                    nc.vector.tensor_scalar_mul(dst, src, gscale)
                else:
                    nc.scalar.mul(dst, src, gscale)

            outT_sb = [per.tile([128, OSH], F32, tag=f"outT{bc}", name=f"outT{bc}")
                       for bc in range(NBC)]

            # ---------------- main loop ----------------
            # o-pair-major: 2 chains (one per o in the pair) live per bc
            # block; G tiles stream in one pair ahead of use.  Each chain
            # drains straight out of PSUM on DVE (no ACT hop).
            NPAIR = OSH // 2
            for pair in range(NPAIR):
                for bc in range(NBC):
                    psts = [ps.tile([128, M], F32, tag="ps",
                                    name=f"ps{pair}_{bc}_{oq}")
                            for oq in range(2)]
                    for kc in range(NKC):
                        st = kc == 0
                        sp = kc == NKC - 1
                        for oq in range(2):
                            o = pair * 2 + oq
                            if bc == 0:
                                lhsT = boot[:, kc * 128:(kc + 1) * 128]
                            else:
                                vcol = (kc * 3 + (bc - 1)) * 128
                                lhsT = vqt[:, vcol:vcol + 128]
                            if pair == 0 and kc == 0:
                                rhs = boot[:, 4 * 128 + oq * M:
                                           4 * 128 + (oq + 1) * M]
                            else:
                                rhs = G_sb[o][:, kc * M:(kc + 1) * M]
                            nc.tensor.matmul(psts[oq][:], lhsT, rhs,
                                             start=st, stop=sp)
                    for oq in range(2):
                        o = pair * 2 + oq
                        jk = junk.tile([128, M], BF16, tag="junk",
                                       name=f"jk{pair}_{bc}_{oq}")
                        nc.vector.scalar_tensor_tensor(
                            out=jk[:], in0=psts[oq][:], scalar=1.0,
                            in1=U_sb[bc][:], op0=ALU.mult, op1=ALU.mult,
                            accum_out=outT_sb[bc][:, o:o + 1])
                    if pair == NPAIR - 1:
                        q = nc.sync if bc % 2 == 0 else nc.scalar
                        q.dma_start(out_d[bc * 128:(bc + 1) * 128, :],
                                    outT_sb[bc][:])

    nc.finalize()
    return nc


def _hat_arms(vals, bd, il):
    """L/R hat arms per (corner, elem, b): L_q = (v-bd[q-1])*il[q-1],
    R_q = (bd[q+1]-v)*il[q]; +-BIG where the arm does not exist."""
    q = np.arange(NQ)
    aL = np.where(q >= 1, il[np.clip(q - 1, 0, NG - 1)], 0.0).astype(np.float32)
    bL = np.where(q >= 1, -bd[np.clip(q - 1, 0, NQ - 1)] * il[np.clip(q - 1, 0, NG - 1)], BIG).astype(np.float32)
    aR = np.where(q <= NG - 1, -il[np.clip(q, 0, NG - 1)], 0.0).astype(np.float32)
    bR = np.where(q <= NG - 1, bd[np.clip(q + 1, 0, NQ - 1)] * il[np.clip(q, 0, NG - 1)], BIG).astype(np.float32)
    L = aL[:, None, None] * vals[None, :, :] + bL[:, None, None]
    R = aR[:, None, None] * vals[None, :, :] + bR[:, None, None]
    return L, R


def _hat_weights(vals, bd, il):
    """Exact fp32 hat weights [(q<=15, elem), b] incl. extrapolation tails."""
    L, R = _hat_arms(vals, bd, il)
    W = np.maximum(np.minimum(L, R), 0.0)
    W[1] = np.minimum(L[1], np.maximum(R[1], 0.0))     # q=1: L un-relu'd
    W[15] = np.minimum(np.maximum(L[15], 0.0), R[15])  # q=15: R un-relu'd
    n = vals.shape[0]
    return W[0:NG].reshape(NG * n, -1)


def _host_prep(x, z, func_parameter, borders, il):
    import ml_dtypes
    x = np.asarray(x, np.float32)
    z = np.asarray(z, np.float32)
    F = np.asarray(func_parameter, np.float32)
    bd = np.asarray(borders, np.float32)
    il = np.asarray(il, np.float32)
    bf = ml_dtypes.bfloat16

    # G_all[o, q*32+iz, p*32+ix] = F[p,q,o,ix,iz]
    K = NQ * IZ
    G_all = np.ascontiguousarray(F.transpose(2, 1, 4, 0, 3)).reshape(OUT, K, K)
    # x-side fold (drop p=16 cols), then z-side fold (drop q=16 rows)
    Ghat = G_all[:, :, 0:M] - np.tile(G_all[:, :, M:K], (1, 1, NG))
    gr = G_all[:, :, M:K].sum(axis=2)                         # [64, 544]
    Gh5 = Ghat.reshape(OUT, NQ, IZ, M)
    Ghh = np.ascontiguousarray(
        (Gh5[:, 0:NG] - Gh5[:, NG:NQ]).reshape(OUT, KF, M))   # [64, 512, 512]
    cbt = Gh5[:, NG].sum(axis=1)                              # [64, 512]
    gr5 = gr.reshape(OUT, NQ, IZ)
    gr2 = (gr5[:, 0:NG] - gr5[:, NG:NQ]).reshape(OUT, KF)     # [64, 512]
    c0 = gr5[:, NG].sum(axis=1)                               # [64]

    # exact fp32 hat weights (shipped bf16) + exact fp32 side correction
    V16 = _hat_weights(z, bd, il)                             # [512k, 512b]
    U16 = _hat_weights(x, bd, il)                             # [512m, 512b]
    side = cbt @ U16 + gr2 @ V16 + c0[:, None]                # [64, 512]

    # vq[128, (kc,bc1..3)*128]: stationary V chunks (bc0 lives in boot)
    vq = np.empty((128, NKC * 3 * 128), np.float32)
    for kc in range(NKC):
        for bc in range(1, NBC):
            c = (kc * 3 + (bc - 1)) * 128
            vq[:, c:c + 128] = \
                V16[kc * 128:(kc + 1) * 128, bc * 128:(bc + 1) * 128]

    # boot: vq bc0 chunks for all kc + kc0 slabs of G0/G1 (per-core G slice
    # differs, so G parts are appended per core below)
    boot_v = np.empty((128, 4 * 128), np.float32)
    for kc in range(NKC):
        boot_v[:, kc * 128:(kc + 1) * 128] = \
            V16[kc * 128:(kc + 1) * 128, 0:128]

    # uu[128b, (bc)*M]: U transposed chunks for stage-2
    uu = np.empty((128, NBC * M), np.float32)
    for bc in range(NBC):
        uu[:, bc * M:(bc + 1) * M] = U16[:, bc * 128:(bc + 1) * 128].T

    SLABS = [(0, 1), (1, 1), (0, 2), (1, 2), (0, 3), (1, 3),
             (2, 0), (3, 0), (2, 1), (3, 1), (2, 2), (3, 2),
             (2, 3), (3, 3)]
    gscale = float(max(np.abs(Ghh).max() / 127.0, 1e-30))
    gmain_all = []
    boot_all = []
    g8_all = []
    for c in range(NCORES):
        Go = Ghh[c * OSH:(c + 1) * OSH]                       # [8, 512, 512]
        gmain32 = np.ascontiguousarray(
            Go.reshape(OSH, NKC, 128, M).transpose(0, 2, 1, 3)
            .reshape(OSH, 128, NKC * M))
        gmain_all.append(gmain32.astype(bf))
        boot = np.empty((128, 4 * 128 + 2 * M), np.float32)
        boot[:, 0:4 * 128] = boot_v
        boot[:, 4 * 128:4 * 128 + M] = gmain32[0][:, 0:M]
        boot[:, 4 * 128 + M:] = gmain32[1][:, 0:M]
        boot_all.append(boot.astype(bf))
        g8 = np.empty((128, 14 * M), np.int8)
        for s, (o, kc) in enumerate(SLABS):
            g8[:, s * M:(s + 1) * M] = np.clip(
                np.round(gmain32[o][:, kc * M:(kc + 1) * M] / gscale),
                -127, 127).astype(np.int8)
        g8_all.append(g8)
    return gmain_all, vq.astype(bf), uu.astype(bf), boot_all, g8_all, gscale, side


def kernel(x, z, func_parameter, borders, inverse_chunk_lengths, _trace=False):
    gmain_all, vq, uu, boot_all, g8_all, gscale, side = _host_prep(
        x, z, func_parameter, borders, inverse_chunk_lengths)

    if _NC_CACHE.get("gscale") != gscale:
        _NC_CACHE["nc"] = _build_nc(gscale)
        _NC_CACHE["gscale"] = gscale
    nc = _NC_CACHE["nc"]

    in_maps = []
    for c in range(NCORES):
        in_maps.append({
            "gmain": gmain_all[c],
            "vq": vq,
            "uu": uu,
            "boot": boot_all[c],
            "g8": g8_all[c],
        })

    res = run_bass_kernel_spmd(nc, in_maps, core_ids=list(range(NCORES)),
                               trace=_trace)
    parts = []
    for c in range(NCORES):
        r = res.results[c]
        parts.append(r["out"].T.astype(np.float32) + side[c * OSH:(c + 1) * OSH])
    out = np.ascontiguousarray(np.concatenate(parts, axis=0).astype(np.float32))
    if _trace:
        return out, res
    return out


# revision 56
# speedup vs baseline: 1.0135x; 1.0135x over previous
"""Trainium2 Bass kernel for nn_BasisFunction2D (2-D basis-function embedding lookup).

Reformulation: data-dependent bilinear interpolation over a 16x16 grid of
per-(ix,iz) tables expressed as dense hat-function interpolation matrices

    V[(q,iz), b] = hat_q(z[iz,b])      (z-side weights, 2 nonzeros per column)
    U[(p,ix), b] = hat_p(x[ix,b])      (x-side weights)

with partition-of-unity folds on BOTH axes, which shrink the contraction to
K=512 (4 full PE chunks) and the free dim to M=512 (one PSUM bank per chain):

    out[o,b] = sum_m C_o[b,m] * U[m,b]  +  side[o,b]
    C_o[b,m] = sum_{k in 512} V[k,b] * Ghh_o[k,m]         (PE, bf16, N=512)
    side = exact fp32 rank-small correction evaluated on the host.

V and U are computed exactly on the host in fp32 and shipped as bf16 (no
on-device build).  The matmul schedule is o-pair-major:

    for pair (2 o's): for bc: for kc: for o' in pair: matmul

so consecutive matmuls alternate PSUM banks (216ns steady cadence) and each
chain drains straight out of PSUM on the DVE (fused multiply-reduce against
U, no ACT hop).  The start is DMA-latency-bound, so a small "boot" tensor
carries exactly what block 0 needs, G0..G3 ship as int8 (half the bytes
through the supply-critical first ~10us; upcast x gscale on the otherwise
idle DVE/ACT engines; the int8 rounding costs ~0.6% extra rel err on half
the outputs, total ~1.2e-2 vs the 2e-2 gate), and G4..G7 ship as bf16 with
relaxed deadlines.  Dummy warmup matmuls ramp the PE p-state (0.65 -> 2.4
GHz needs ~3us of continuous busy) while the first chunks land.
"""

import numpy as np

import concourse.bass as bass
import concourse.bacc as bacc_mod
import concourse.tile as tile
from concourse import mybir
from concourse.bass_utils import run_bass_kernel_spmd

F32 = mybir.dt.float32
BF16 = mybir.dt.bfloat16
ALU = mybir.AluOpType

NCORES = 8
NG = 16            # grid bins
NQ = 17            # grid corners per axis
IX = 32
IZ = 32
OUT = 64
B = 512
OSH = OUT // NCORES          # outputs per core = 8
KF = NG * IZ                 # 512 folded contraction rows (q<=15, iz)
M = NG * IX                  # 512 folded free cols (p<=15, ix)
BIG = 1e30
NBC = B // 128               # 4 batch chunks
NKC = 4                      # contraction chunks of 128
NWARM = 8            # PE warmup matmuls (p-state ramp + DMA cover)

_NC_CACHE = {}


def _build_nc(gscale):
    """Build the single-core Bass/Tile program (identical across cores).
    gscale is the int8 dequant scale for the g8 slabs (compile-time const)."""
    nc = bacc_mod.Bacc(None, target_bir_lowering=False)
    gmain_d = nc.dram_tensor("gmain", [OSH, 128, NKC * M], BF16, kind="ExternalInput")
    # vq: V chunks for bc1..3 only (bc0 lives in boot), kc-major:
    # col ((kc*3 + (bc-1)) * 128)
    vq_d = nc.dram_tensor("vq", [128, NKC * 3 * 128], BF16, kind="ExternalInput")
    uu_d = nc.dram_tensor("uu", [128, NBC * M], BF16, kind="ExternalInput")
    # boot: [vq bc0 kc0..3 (4*128) | G0 kc0 (512) | G1 kc0 (512)] — one small
    # leading DMA so block 0 can start before the big tiles land.
    boot_d = nc.dram_tensor("boot", [128, 4 * 128 + 2 * M], BF16,
                            kind="ExternalInput")
    # g8: int8 slabs for the DMA-supply-critical tiles G0..G3 (kc0 of G0/G1
    # lives in boot as bf16).  Slab order by consumption deadline:
    # [G0kc1 G1kc1 G0kc2 G1kc2 G0kc3 G1kc3 G2kc0 G3kc0 ... G2kc3 G3kc3]
    g8_d = nc.dram_tensor("g8", [128, 14 * M], mybir.dt.int8,
                          kind="ExternalInput")
    out_d = nc.dram_tensor("out", [B, OSH], F32, kind="ExternalOutput")

    with tile.TileContext(nc) as tc:
        with (
            tc.tile_pool(name="per", bufs=1) as per,       # persistent tiles
            tc.tile_pool(name="junk", bufs=2) as junk,     # stt mandatory outs
            tc.tile_pool(name="ps", bufs=8, space="PSUM") as ps,
        ):
            # ---------------- PE warmup ----------------
            # Dependency-free dummy matmuls ramp the PE p-state while the
            # first input DMAs land.  The memset rides the otherwise-idle
            # DVE queue.
            wt = per.tile([128, B], BF16, tag="warm", name="wt")
            nc.vector.memset(wt[:], 0.0)
            wps = ps.tile([128, B], F32, tag="ps", name="wps")
            for _ in range(NWARM):
                nc.tensor.matmul(wps[:], wt[:, 0:128], wt[:], start=True, stop=True)
            for _ in range(4):
                nc.tensor.matmul(wps[:, 0:128], wt[:, 0:128], wt[:, 0:128],
                                 start=True, stop=True)

            # ---------------- input loads ----------------
            # Start-critical chunks lead the sync/scalar queues (gpsimd's
            # queue is stalled by framework drains); first matmul needs only
            # vq[:, :128] + G0's first half.  U tiles ride gpsimd (needed
            # ~5us later than the G stream).
            vqt = per.tile([128, NKC * 3 * 128], BF16, tag="vq", name="vqt")
            U_sb = [per.tile([128, M], BF16, tag=f"U{bc}", name=f"U{bc}")
                    for bc in range(NBC)]
            G_sb = [per.tile([128, NKC * M], BF16, tag=f"G{o}", name=f"G{o}")
                    for o in range(OSH)]

            # ordered by consumption deadline; g8 (int8, half the bytes)
            # carries the supply-critical G0..G3 slabs and is upcast on the
            # DVE (early slabs, before drains start) and ACT (late slabs).
            boot = per.tile([128, 4 * 128 + 2 * M], BF16, tag="boot", name="boot")
            g8t = per.tile([128, 14 * M], mybir.dt.int8, tag="g8", name="g8t")
            VB = 3 * 128
            # late G tiles ride the gpsimd queue (its framework drains clear
            # by ~16us, well before the pair-2/3 deadlines), freeing sync/
            # scalar bandwidth for the supply-critical first 12us.
            nc.scalar.dma_start(g8t[:, 0:4 * M], g8_d[:, 0:4 * M])
            nc.sync.dma_start(boot[:], boot_d[:, :])
            nc.scalar.dma_start(g8t[:, 4 * M:6 * M], g8_d[:, 4 * M:6 * M])
            nc.sync.dma_start(vqt[:, 0:2 * VB], vq_d[:, 0:2 * VB])
            nc.sync.dma_start(vqt[:, 2 * VB:4 * VB], vq_d[:, 2 * VB:4 * VB])
            nc.sync.dma_start(g8t[:, 6 * M:10 * M], g8_d[:, 6 * M:10 * M])
            nc.sync.dma_start(g8t[:, 10 * M:14 * M], g8_d[:, 10 * M:14 * M])
            nc.sync.dma_start(U_sb[0][:], uu_d[:, 0:M])
            nc.scalar.dma_start(U_sb[1][:], uu_d[:, M:2 * M])
            nc.scalar.dma_start(U_sb[2][:], uu_d[:, 2 * M:3 * M])
            nc.scalar.dma_start(U_sb[3][:], uu_d[:, 3 * M:4 * M])
            nc.sync.dma_start(G_sb[4][:], gmain_d[4])
            nc.scalar.dma_start(G_sb[5][:], gmain_d[5])
            nc.sync.dma_start(G_sb[6][:], gmain_d[6])
            nc.scalar.dma_start(G_sb[7][:], gmain_d[7])

            # int8 -> bf16 upcasts (x gscale); slab s holds (o_loc, kc):
            SLABS = [(0, 1), (1, 1), (0, 2), (1, 2), (0, 3), (1, 3),
                     (2, 0), (3, 0), (2, 1), (3, 1), (2, 2), (3, 2),
                     (2, 3), (3, 3)]
            for s, (o, kc) in enumerate(SLABS):
                dst = G_sb[o][:, kc * M:(kc + 1) * M]
                src = g8t[:, s * M:(s + 1) * M]
                if s < 4:
                    nc.vector.tensor_scalar_mul(dst, src, gscale)
                else:
                    nc.scalar.mul(dst, src, gscale)

            outT_sb = [per.tile([128, OSH], F32, tag=f"outT{bc}", name=f"outT{bc}")
                       for bc in range(NBC)]

            # ---------------- main loop ----------------
            # o-pair-major: 2 chains (one per o in the pair) live per bc
            # block; G tiles stream in one pair ahead of use.  Each chain
            # drains straight out of PSUM on DVE (no ACT hop).
            NPAIR = OSH // 2
            for pair in range(NPAIR):
                for bc in range(NBC):
                    psts = [ps.tile([128, M], F32, tag="ps",
                                    name=f"ps{pair}_{bc}_{oq}")
                            for oq in range(2)]
                    for kc in range(NKC):
                        st = kc == 0
                        sp = kc == NKC - 1
                        for oq in range(2):
                            o = pair * 2 + oq
                            if bc == 0:
                                lhsT = boot[:, kc * 128:(kc + 1) * 128]
                            else:
                                vcol = (kc * 3 + (bc - 1)) * 128
                                lhsT = vqt[:, vcol:vcol + 128]
                            if pair == 0 and kc == 0:
                                rhs = boot[:, 4 * 128 + oq * M:
                                           4 * 128 + (oq + 1) * M]
                            else:
                                rhs = G_sb[o][:, kc * M:(kc + 1) * M]
                            nc.tensor.matmul(psts[oq][:], lhsT, rhs,
                                             start=st, stop=sp)
                    for oq in range(2):
                        o = pair * 2 + oq
                        jk = junk.tile([128, M], BF16, tag="junk",
                                       name=f"jk{pair}_{bc}_{oq}")
                        nc.vector.scalar_tensor_tensor(
                            out=jk[:], in0=psts[oq][:], scalar=1.0,
                            in1=U_sb[bc][:], op0=ALU.mult, op1=ALU.mult,
                            accum_out=outT_sb[bc][:, o:o + 1])
                    if pair == NPAIR - 1:
                        q = nc.sync if bc % 2 == 0 else nc.scalar
                        q.dma_start(out_d[bc * 128:(bc + 1) * 128, :],
                                    outT_sb[bc][:])

    nc.finalize()
    return nc


def _hat_arms(vals, bd, il):
    """L/R hat arms per (corner, elem, b): L_q = (v-bd[q-1])*il[q-1],
    R_q = (bd[q+1]-v)*il[q]; +-BIG where the arm does not exist."""
    q = np.arange(NQ)
    aL = np.where(q >= 1, il[np.clip(q - 1, 0, NG - 1)], 0.0).astype(np.float32)
    bL = np.where(q >= 1, -bd[np.clip(q - 1, 0, NQ - 1)] * il[np.clip(q - 1, 0, NG - 1)], BIG).astype(np.float32)
    aR = np.where(q <= NG - 1, -il[np.clip(q, 0, NG - 1)], 0.0).astype(np.float32)
    bR = np.where(q <= NG - 1, bd[np.clip(q + 1, 0, NQ - 1)] * il[np.clip(q, 0, NG - 1)], BIG).astype(np.float32)
    L = aL[:, None, None] * vals[None, :, :] + bL[:, None, None]
    R = aR[:, None, None] * vals[None, :, :] + bR[:, None, None]
    return L, R


def _hat_weights(vals, bd, il):
    """Exact fp32 hat weights [(q<=15, elem), b] incl. extrapolation tails."""
    L, R = _hat_arms(vals, bd, il)
    W = np.maximum(np.minimum(L, R), 0.0)
    W[1] = np.minimum(L[1], np.maximum(R[1], 0.0))     # q=1: L un-relu'd
    W[15] = np.minimum(np.maximum(L[15], 0.0), R[15])  # q=15: R un-relu'd
    n = vals.shape[0]
    return W[0:NG].reshape(NG * n, -1)


def _host_prep(x, z, func_parameter, borders, il):
    import ml_dtypes
    x = np.asarray(x, np.float32)
    z = np.asarray(z, np.float32)
    F = np.asarray(func_parameter, np.float32)
    bd = np.asarray(borders, np.float32)
    il = np.asarray(il, np.float32)
    bf = ml_dtypes.bfloat16

    # G_all[o, q*32+iz, p*32+ix] = F[p,q,o,ix,iz]
    K = NQ * IZ
    G_all = np.ascontiguousarray(F.transpose(2, 1, 4, 0, 3)).reshape(OUT, K, K)
    # x-side fold (drop p=16 cols), then z-side fold (drop q=16 rows)
    Ghat = G_all[:, :, 0:M] - np.tile(G_all[:, :, M:K], (1, 1, NG))
    gr = G_all[:, :, M:K].sum(axis=2)                         # [64, 544]
    Gh5 = Ghat.reshape(OUT, NQ, IZ, M)
    Ghh = np.ascontiguousarray(
        (Gh5[:, 0:NG] - Gh5[:, NG:NQ]).reshape(OUT, KF, M))   # [64, 512, 512]
    cbt = Gh5[:, NG].sum(axis=1)                              # [64, 512]
    gr5 = gr.reshape(OUT, NQ, IZ)
    gr2 = (gr5[:, 0:NG] - gr5[:, NG:NQ]).reshape(OUT, KF)     # [64, 512]
    c0 = gr5[:, NG].sum(axis=1)                               # [64]

    # exact fp32 hat weights (shipped bf16) + exact fp32 side correction
    V16 = _hat_weights(z, bd, il)                             # [512k, 512b]
    U16 = _hat_weights(x, bd, il)                             # [512m, 512b]
    side = cbt @ U16 + gr2 @ V16 + c0[:, None]                # [64, 512]

    # vq[128, (kc,bc1..3)*128]: stationary V chunks (bc0 lives in boot)
    vq = np.empty((128, NKC * 3 * 128), np.float32)
    for kc in range(NKC):
        for bc in range(1, NBC):
            c = (kc * 3 + (bc - 1)) * 128
            vq[:, c:c + 128] = \
                V16[kc * 128:(kc + 1) * 128, bc * 128:(bc + 1) * 128]

    # boot: vq bc0 chunks for all kc + kc0 slabs of G0/G1 (per-core G slice
    # differs, so G parts are appended per core below)
    boot_v = np.empty((128, 4 * 128), np.float32)
    for kc in range(NKC):
        boot_v[:, kc * 128:(kc + 1) * 128] = \
            V16[kc * 128:(kc + 1) * 128, 0:128]

    # uu[128b, (bc)*M]: U transposed chunks for stage-2
    uu = np.empty((128, NBC * M), np.float32)
    for bc in range(NBC):
        uu[:, bc * M:(bc + 1) * M] = U16[:, bc * 128:(bc + 1) * 128].T

    SLABS = [(0, 1), (1, 1), (0, 2), (1, 2), (0, 3), (1, 3),
             (2, 0), (3, 0), (2, 1), (3, 1), (2, 2), (3, 2),
             (2, 3), (3, 3)]
    gscale = float(max(np.abs(Ghh).max() / 127.0, 1e-30))
    gmain_all = []
    boot_all = []
    g8_all = []
    for c in range(NCORES):
        Go = Ghh[c * OSH:(c + 1) * OSH]                       # [8, 512, 512]
        gmain32 = np.ascontiguousarray(
            Go.reshape(OSH, NKC, 128, M).transpose(0, 2, 1, 3)
            .reshape(OSH, 128, NKC * M))
        gmain_all.append(gmain32.astype(bf))
        boot = np.empty((128, 4 * 128 + 2 * M), np.float32)
        boot[:, 0:4 * 128] = boot_v
        boot[:, 4 * 128:4 * 128 + M] = gmain32[0][:, 0:M]
        boot[:, 4 * 128 + M:] = gmain32[1][:, 0:M]
        boot_all.append(boot.astype(bf))
        g8 = np.empty((128, 14 * M), np.int8)
        for s, (o, kc) in enumerate(SLABS):
            g8[:, s * M:(s + 1) * M] = np.clip(
                np.round(gmain32[o][:, kc * M:(kc + 1) * M] / gscale),
                -127, 127).astype(np.int8)
        g8_all.append(g8)
    return gmain_all, vq.astype(bf), uu.astype(bf), boot_all, g8_all, gscale, side


def kernel(x, z, func_parameter, borders, inverse_chunk_lengths, _trace=False):
    gmain_all, vq, uu, boot_all, g8_all, gscale, side = _host_prep(
        x, z, func_parameter, borders, inverse_chunk_lengths)

    if _NC_CACHE.get("gscale") != gscale:
        _NC_CACHE["nc"] = _build_nc(gscale)
        _NC_CACHE["gscale"] = gscale
    nc = _NC_CACHE["nc"]

    in_maps = []
    for c in range(NCORES):
        in_maps.append({
            "gmain": gmain_all[c],
            "vq": vq,
            "uu": uu,
            "boot": boot_all[c],
            "g8": g8_all[c],
        })

    res = run_bass_kernel_spmd(nc, in_maps, core_ids=list(range(NCORES)),
                               trace=_trace)
    parts = []
    for c in range(NCORES):
        r = res.results[c]
        parts.append(r["out"].T.astype(np.float32) + side[c * OSH:(c + 1) * OSH])
    out = np.ascontiguousarray(np.concatenate(parts, axis=0).astype(np.float32))
    if _trace:
        return out, res
    return out
